# revision 31
# baseline (speedup 1.0000x reference)
"""AttentionBlock (GroupNorm + single-head self-attention + proj + residual)
on 8 TRN2 NeuronCores. Data-parallel over batch: core i handles sample i.

Reference computation per sample (C=256, H=W=64, N=H*W=4096, G=32 groups):
  h    = groupnorm(x) * gamma + beta
  qkv  = w_qkv @ h + b_qkv              (1x1 conv == channel matmul)
  attn = softmax(q^T k / sqrt(C))       (N x N, never materialized in HBM)
  out  = x + w_proj @ (v @ attn^T) + b_proj

v10 structure (vs the 257us baseline):
  - w_proj folded into the v projection on the host: vp = (w_proj@w_v) h,
    so attn@v directly produces projected channels; the 32 proj matmuls and
    the bf16 att tiles disappear. out = x + (E vp)/den + bp_eff.
  - GroupNorm folded into the qkv weights on device (w' = w * sc per input
    channel), so x casts to fp8 on arrival and no h tensor is materialized.
  - bias algebra: k needs NO bias (a per-column score offset cancels in
    softmax); vp needs NO tensor bias (a constant vp offset rides through
    softmax into the output bias: bp_eff = bproj + wproj@bv + W_vp@sh).
    Only q keeps a [P,1] bias.
  - scores live in [P, 2, NBLK] two-bank psum pair-tiles, so the softmax
    exp is ONE 1068ns ACT instruction per m-pair (vs 2x679) and ALL exps
    fit on ACT (~17.1us/block ~ the PE's 17.2us pair stream). The DVE
    stays nearly idle -> no concurrent ACT+DVE clock throttle (which
    drops the chip from 2.4 to 2.0GHz and can latch), no idle wake-ups.
  - PSUM: score pairs 2x2 banks + av0 1 + av1 2 + den 1 = 8. av0 is
    single-buffered: its pair 0-2 matmuls are deferred to iterations
    13-15 (accumulation order is commutative; start flag rides pair 3,
    stop rides the deferred pair 2), clearing the previous block's
    divide without a bubble. qkv borrows score pair-tiles.
  - q projections for blocks 2..7 are deferred into attention blocks
    1..6 (copies on the idle DVE); divide/bias/residual also DVE.
  - x stays resident in SBUF for the residual (no 4MB re-read).
"""

import sys

for _p in ("/opt/trn_rl_repo", "/opt/pypackages"):
    if _p not in sys.path:
        sys.path.append(_p)

from contextlib import ExitStack

import numpy as np

import concourse.bass as bass
import concourse.tile as tile
from concourse import bacc, mybir
from concourse._compat import with_exitstack

B, C, H, W = 8, 256, 64, 64
N = H * W          # 4096
G = 32             # groups
GS = C // G        # 8 channels per group
EPS = 1e-5
P = 128
NCT = C // P       # 2 channel tiles
NBLK = 512         # attention n-block width
NB = N // NBLK     # 8
NM = N // P        # 32 m-tiles
NPAIR = NM // 2    # 16 m-pairs per block
NGP = NB * NPAIR   # 128 global pairs
SCALE = 1.0 / np.sqrt(np.float32(C))  # 1/16
WARMUP_MM = 56      # fp32 gmat matmuls to keep PE's HAM clock-gate warm

F32 = mybir.dt.float32
BF16 = mybir.dt.bfloat16
FP8 = mybir.dt.float8e4
DR = mybir.MatmulPerfMode.DoubleRow
AF = mybir.ActivationFunctionType
ALU = mybir.AluOpType


def _group_mat() -> np.ndarray:
    """A[c, c'] = 1/GS if c and c' are in the same group (within a 128-chan
    tile); A^T @ t group-averages per-channel stats in one PE matmul."""
    a = np.zeros((P, P), np.float32)
    for g in range(P // GS):
        a[g * GS:(g + 1) * GS, g * GS:(g + 1) * GS] = 1.0 / GS
    return a


def _col(ap_1d, lo, hi):
    """Slice a 1-D DRAM AP into a [hi-lo, 1] AP (partition dim x 1)."""
    sl = ap_1d[lo:hi]
    return bass.AP(tensor=sl.tensor, offset=sl.offset, ap=[*sl.ap, [1, 1]])


@with_exitstack
def emit_kernel(ctx: ExitStack, tc: tile.TileContext, out_d, x_d, wqkvT_d,
                bqkv_d, bproj_d, gamma_d, beta_d, gmat_d):
    nc = tc.nc

    big = ctx.enter_context(tc.tile_pool(name="big", bufs=1))
    small = ctx.enter_context(tc.tile_pool(name="small", bufs=1))
    work = ctx.enter_context(tc.tile_pool(name="work", bufs=3))
    workd = ctx.enter_context(tc.tile_pool(name="workd", bufs=3))
    work2 = ctx.enter_context(tc.tile_pool(name="work2", bufs=3))
    tdiv = ctx.enter_context(tc.tile_pool(name="tdiv", bufs=4))
    stage = ctx.enter_context(tc.tile_pool(name="stage", bufs=4))
    ps_sp = ctx.enter_context(tc.tile_pool(name="ps_sp", bufs=2, space="PSUM"))
    ps_av0 = ctx.enter_context(tc.tile_pool(name="ps_av0", bufs=1, space="PSUM"))
    ps_av1 = ctx.enter_context(tc.tile_pool(name="ps_av1", bufs=2, space="PSUM"))
    ps_sum = ctx.enter_context(tc.tile_pool(name="ps_sum", bufs=1, space="PSUM"))

    def ps_pair(name):
        return ps_sp.tile([P, 2, NBLK], F32, tag="sp", name=name)

    # ---- gmat first: its DVE copy feeds PE warmup matmuls that keep the
    # HAM clock-gate warm while x loads / groupnorm stats run ----
    gmat_f = small.tile([P, P], F32, tag="gmatf")
    nc.sync.dma_start(gmat_f, gmat_d[:, :])
    gmat_sb = small.tile([P, P], F32, tag="gmat")
    nc.vector.tensor_copy(gmat_sb, gmat_f)
    for w in range(WARMUP_MM):
        pw = ps_pair(f"warm{w}")
        nc.tensor.matmul(pw[:, 0, 0:P], lhsT=gmat_sb, rhs=gmat_sb,
                         start=True, stop=True)

    # ---- constants / weights to SBUF ----
    gamma_t = []
    beta_t = []
    bp_t = []
    for ct in range(NCT):
        gt = small.tile([P, 1], F32, tag=f"gamma{ct}")
        nc.sync.dma_start(gt, _col(gamma_d, ct * P, (ct + 1) * P))
        gamma_t.append(gt)
        bt = small.tile([P, 1], F32, tag=f"beta{ct}")
        nc.sync.dma_start(bt, _col(beta_d, ct * P, (ct + 1) * P))
        beta_t.append(bt)
        t = small.tile([P, 1], F32, tag=f"bp{ct}")
        nc.sync.dma_start(t, _col(bproj_d, ct * P, (ct + 1) * P))
        bp_t.append(t)
    bq_in = []
    for o in range(NCT):  # only q's bias survives the softmax algebra
        t = small.tile([P, 1], F32, tag=f"bq{o}")
        nc.sync.dma_start(t, _col(bqkv_d, o * P, (o + 1) * P))
        bq_in.append(t)

    eps_t = small.tile([P, 1], F32, tag="eps")
    nc.vector.memset(eps_t, float(EPS))
    # preload the Sqrt act table while the engines boot (Sqrt and Exp live
    # in different table sets; each implicit load costs 1.28us on ACT)
    sqrt_dummy = small.tile([P, 1], F32, tag="sqrt_dummy")
    nc.scalar.activation(sqrt_dummy, eps_t, AF.Sqrt, bias=eps_t)

    # ---- load x (3-way split across the SP / GPSIMD / ACT DGE queues);
    # bn_stats (DVE) + fp8 cast (ACT) interleave with chunk arrival. The
    # scalar queue takes the LAST 5 chunk indices so stats emission order
    # matches arrival order, and wqf (0.79MB, needed only at fold time
    # ~30us) loads after them. ----
    x_sb = []
    stats_t = []
    for ct in range(NCT):
        xt = big.tile([P, N], F32, tag=f"x{ct}", name=f"x{ct}")
        x_sb.append(xt)
        stats_t.append(small.tile([P, NB, 6], F32, tag=f"bnst{ct}",
                                  name=f"bnst{ct}"))
    x8 = big.tile([P, 2, N], FP8, tag="x8")
    qs_order = ([nc.sync, nc.gpsimd] * 6)[:11] + [nc.scalar] * 5
    for j in range(NB):
        for ct in range(NCT):
            eng = qs_order[2 * j + ct]
            eng.dma_start(x_sb[ct][:, j * NBLK:(j + 1) * NBLK],
                          x_d[ct * P:(ct + 1) * P, j * NBLK:(j + 1) * NBLK])
    wqf = small.tile([P, 2, 3 * C], F32, tag="wqkvTf", name="wqf")
    nc.scalar.dma_start(wqf, wqkvT_d[:, :, :])
    for j in range(NB):
        for ct in range(NCT):
            csl = slice(j * NBLK, (j + 1) * NBLK)
            nc.vector.bn_stats(stats_t[ct][:, j, :], x_sb[ct][:, csl])
            nc.scalar.copy(x8[:, ct, csl], x_sb[ct][:, csl])

    # ---- GN stats -> per-channel scale/shift (h = x*sc + sh) ----
    scale_sh = []
    for ct in range(NCT):
        mv = small.tile([P, 2], F32, tag=f"mv{ct}")
        nc.vector.bn_aggr(mv, stats_t[ct])
        # t = [mean_c, E[x^2]_c]
        t = small.tile([P, 2], F32, tag=f"t{ct}")
        nc.vector.tensor_copy(t[:, 0:1], mv[:, 0:1])
        nc.vector.tensor_mul(t[:, 1:2], mv[:, 0:1], mv[:, 0:1])
        nc.vector.tensor_add(t[:, 1:2], t[:, 1:2], mv[:, 1:2])
        # group-average + broadcast back to channels via PE
        psg = ps_pair(f"psg{ct}")
        nc.tensor.matmul(psg[:, 0, 0:2], lhsT=gmat_sb, rhs=t,
                         start=True, stop=True)
        g_sb = small.tile([P, 2], F32, tag=f"g{ct}")
        nc.vector.tensor_copy(g_sb, psg[:, 0, 0:2])
        # scale = gamma * rsqrt(var + eps);  shift = beta - group_mean * scale
        tmp = small.tile([P, 1], F32, tag=f"tmp{ct}")
        sc = small.tile([P, 1], F32, tag=f"sc{ct}")
        sh = small.tile([P, 1], F32, tag=f"sh{ct}")
        nc.vector.tensor_mul(tmp, g_sb[:, 0:1], g_sb[:, 0:1])
        nc.vector.tensor_tensor(tmp, g_sb[:, 1:2], tmp, ALU.subtract)  # var
        nc.scalar.activation(tmp, tmp, AF.Sqrt, bias=eps_t)
        nc.vector.reciprocal(tmp, tmp)                                 # rstd
        nc.vector.tensor_mul(sc, tmp, gamma_t[ct])
        nc.vector.tensor_mul(tmp, g_sb[:, 0:1], sc)
        nc.vector.tensor_tensor(sh, beta_t[ct], tmp, ALU.subtract)
        scale_sh.append((sc, sh))
    # switch the ACT table back to Exp now, off the critical path, so the
    # first softmax exp doesn't pay the 1.28us implicit load
    nc.scalar.activation(sqrt_dummy, eps_t, AF.Exp, scale=1.0)

    # ---- fold GN into the weights: wq8[c,:] = wqf[c,:] * sc[c] in fp8;
    # q bias' = W_q @ sh + b_q and bp_eff += W_vp @ sh via tiny matmuls ----
    wq8 = small.tile([P, 2, 3 * C], FP8, tag="wqkvT8", name="wq8")
    for ct in range(NCT):
        nc.vector.tensor_scalar(wq8[:, ct, :], wqf[:, ct, :], scale_sh[ct][0],
                                None, op0=ALU.mult)
    bias_q = []
    bp_eff = []
    for o in range(NCT):
        psb = ps_pair(f"psbq{o}")
        for ct in range(NCT):
            nc.tensor.matmul(psb[:, 0, 0:1],
                             lhsT=wqf[:, ct, o * P:(o + 1) * P],
                             rhs=scale_sh[ct][1], start=(ct == 0),
                             stop=(ct == NCT - 1))
        bj = small.tile([P, 1], F32, tag=f"biasq{o}")
        nc.vector.tensor_add(bj, psb[:, 0, 0:1], bq_in[o])
        bias_q.append(bj)
    for o in range(NCT):
        psb = ps_pair(f"psbp{o}")
        for ct in range(NCT):
            nc.tensor.matmul(
                psb[:, 0, 0:1],
                lhsT=wqf[:, ct, 2 * C + o * P:2 * C + (o + 1) * P],
                rhs=scale_sh[ct][1], start=(ct == 0), stop=(ct == NCT - 1))
        bj = small.tile([P, 1], F32, tag=f"bpe{o}")
        nc.vector.tensor_add(bj, psb[:, 0, 0:1], bp_t[o])
        bp_eff.append(bj)

    # ---- qkv projections (GN pre-folded, so rhs is x8 directly). q/k land
    # in fp8 [128, 2, N] (channel-half on the middle dim) and vp in fp8
    # m-pair-interleaved [128, 2, 272] tiles so the attention matmuls can use
    # fp8 DoubleRow (2 values/PE-cell -> one matmul contracts 256). vp is the
    # w_proj-fused v projection; col 256 = ones (softmax denominators). ----
    q2 = big.tile([P, 2, N], FP8, tag="q2")
    k2 = big.tile([P, 2, N], FP8, tag="k2")
    VTW = 272
    vt_lo = big.tile([P, NM // 4, 2, VTW], FP8, tag="vt0", name="vt_lo")
    vt_hi = big.tile([P, NM // 4, 2, VTW], FP8, tag="vt1", name="vt_hi")
    # pre-fill the ones columns once (strided memsets, off-critical-path)
    nc.gpsimd.memset(vt_lo[:, :, :, C:C + 1], 1.0)
    nc.gpsimd.memset(vt_hi[:, :, :, C:C + 1], 1.0)

    def vt2(pair):
        return (vt_lo[:, pair] if pair < NM // 4
                else vt_hi[:, pair - NM // 4])

    def emit_q_blk(blk, only_o=None, on_act=False):
        """q projection for block blk: 2 matmuls + 2 biased copies.
        Deferrable (per channel-half) to just before block blk needs q2."""
        bsl = slice(blk * NBLK, (blk + 1) * NBLK)
        ps = ps_pair(f"q{blk}_{only_o}")
        for o in range(NCT):
            if only_o is not None and o != only_o:
                continue
            half = 0 if only_o is not None else o
            nc.tensor.matmul(
                ps[:, half], lhsT=wq8[:, :, o * P:(o + 1) * P],
                rhs=x8[:, :, bsl], start=True, stop=True, perf_mode=DR)
            if on_act:
                nc.scalar.activation(q2[:, o, bsl], ps[:, half], AF.Identity,
                                     bias=bias_q[o], scale=1.0)
            else:
                nc.vector.tensor_scalar_add(q2[:, o, bsl], ps[:, half],
                                            bias_q[o])

    def emit_kv_blk(blk):
        """k + vp projections for block blk (biases cancel / fold away, so
        all copies are plain psum->fp8 casts; DVE takes most, ACT the
        rest within its exp-stream slack)."""
        bsl = slice(blk * NBLK, (blk + 1) * NBLK)
        psk = ps_pair(f"k{blk}")
        for o in range(NCT):
            nc.tensor.matmul(
                psk[:, o], lhsT=wq8[:, :, C + o * P:C + (o + 1) * P],
                rhs=x8[:, :, bsl], start=True, stop=True, perf_mode=DR)
            nc.vector.tensor_copy(k2[:, o, bsl], psk[:, o])
        psv = ps_pair(f"v{blk}")
        for i in range(4):
            m = 4 * blk + i
            sl = psv[:, i // 2, (i % 2) * C:(i % 2) * C + C]
            nc.tensor.matmul(
                sl, lhsT=x8[:, :, m * P:(m + 1) * P],
                rhs=wq8[:, :, 2 * C:3 * C],
                start=True, stop=True, perf_mode=DR)
            dst = vt2(m // 2)[:, m % 2]
            if m % 4 == 0:
                nc.scalar.copy(dst[:, 0:C], sl)
            else:
                nc.vector.tensor_copy(dst[:, 0:C], sl)

    # ---- softmax divide + output helpers (all on the idle DVE/Pool) ----
    def emit_div_a(pend):
        pav0, pav1, psum, nb = pend
        sums_sb = work2.tile([1, NBLK], F32, tag="sums")
        nc.vector.tensor_copy(sums_sb, psum)
        bc2 = work2.tile([P, NBLK], F32, tag="bc2")
        nc.gpsimd.partition_broadcast(bc2, sums_sb)
        bc_sb = work2.tile([P, NBLK], F32, tag="bc")
        nc.vector.reciprocal_approx_fast(bc_sb, bc2)
        return bc_sb

    def emit_div_b(pend, bc_sb, o):
        """One output channel-half: divide + bias + residual + store."""
        pav = pend[o]
        nb = pend[3]
        nsl = slice(nb * NBLK, (nb + 1) * NBLK)
        t = tdiv.tile([P, NBLK], F32, tag="t")
        nc.vector.tensor_mul(t, pav, bc_sb)
        st = stage.tile([P, NBLK], F32, tag="st")
        nc.vector.scalar_tensor_tensor(st, t, bp_eff[o], x_sb[o][:, nsl],
                                       op0=ALU.add, op1=ALU.add)
        eng = nc.sync if o == 0 else nc.gpsimd
        eng.dma_start(out_d[o * P:(o + 1) * P, nsl], st)

    # ---- global software-pipelined attention loop over gp = nb*16 + p.
    # Iteration gp emits: the single ACT exp for pair gp+1 (runs while the
    # PE works), score matmuls for gp+2 into a fresh pair-tile (half 0
    # early, half 1 after the avs), and the av matmuls for gp (exp'd last
    # iteration -> a full pair-period of exp slack). av0 is single-
    # buffered: pairs 0-2 of each block nb>=1 are deferred to iterations
    # 13-15, after the previous block's av0 divide has read the bank. ----
    ps_m = {}
    e2_pend = {}
    blk_tiles = {}

    def emit_scores(gp, half):
        if gp >= NGP:
            return
        nb, p = divmod(gp, NPAIR)
        m = 2 * p + half
        if half == 0:
            ps_m[gp] = ps_pair(f"s{gp}")
        nc.tensor.matmul(ps_m[gp][:, half],
                         lhsT=k2[:, :, m * P:(m + 1) * P],
                         rhs=q2[:, :, nb * NBLK:(nb + 1) * NBLK],
                         start=True, stop=True, perf_mode=DR)

    def emit_exps(gp):
        if gp >= NGP:
            return
        nb, p = divmod(gp, NPAIR)
        pool = workd if (nb >= 1 and p <= 2) else work
        e2 = pool.tile([P, 2, NBLK], FP8, tag="e", name=f"e{gp}")
        nc.scalar.activation(e2, ps_m.pop(gp), AF.Exp, scale=float(SCALE))
        e2_pend[gp] = e2

    def emit_av0(gp, first, last):
        nb, p = divmod(gp, NPAIR)
        pav0 = blk_tiles[nb][0]
        nc.tensor.matmul(pav0, lhsT=vt2(p)[:, :, 0:P], rhs=e2_pend[gp],
                         start=first, stop=last, perf_mode=DR)

    def emit_av1_den(gp, with_den=True):
        nb, p = divmod(gp, NPAIR)
        _, pav1, psum = blk_tiles[nb]
        first, last = (p == 0), (p == NPAIR - 1)
        vtp = vt2(p)
        nc.tensor.matmul(pav1, lhsT=vtp[:, :, P:2 * P], rhs=e2_pend[gp],
                         start=first, stop=last, perf_mode=DR)
        if with_den:
            nc.tensor.matmul(psum, lhsT=vtp[:, :, 2 * P:2 * P + 1],
                             rhs=e2_pend[gp], start=first, stop=last,
                             perf_mode=DR)

    def e2_done(gp):
        e2_pend.pop(gp)

    def new_blk_tiles(nb):
        pav0 = ps_av0.tile([P, NBLK], F32, tag="av0", name=f"av0_{nb}")
        pav1 = ps_av1.tile([P, NBLK], F32, tag="av1", name=f"av1_{nb}")
        psum = ps_sum.tile([1, NBLK], F32, tag="sum", name=f"sum_{nb}")
        blk_tiles[nb] = (pav0, pav1, psum)

    # Fused phase gating: kv block b unlocks k2 m-tiles < 4(b+1) and vt
    # pairs < 2(b+1); at iteration gp of block 0 the scores reach m-tile
    # 2*gp+5 and the avs read vt pair gp, both covered once kv blocks
    # <= gp/2 + 2 are in.
    emit_q_blk(0, on_act=True)
    emit_kv_blk(0)
    emit_kv_blk(1)
    emit_scores(0, 0)
    emit_scores(0, 1)
    emit_exps(0)
    emit_scores(1, 0)
    emit_scores(1, 1)
    emit_q_blk(1, on_act=True)

    state = {"pend": None, "bc_prev": None}
    for gp in range(NGP):
        nb, p = divmod(gp, NPAIR)
        if nb not in blk_tiles:
            new_blk_tiles(nb)
        if nb == 0:
            if p % 2 == 0 and 2 + p // 2 < NB:
                emit_kv_blk(2 + p // 2)
            if p == 12:
                emit_q_blk(2)
        defer_av0 = (nb >= 1 and p <= 2)
        emit_exps(gp + 1)
        emit_scores(gp + 2, 0)
        if nb > 0:
            if p == 3:
                emit_div_b(state["pend"], state["bc_prev"], 0)
            if p == 4:
                emit_div_b(state["pend"], state["bc_prev"], 1)
            if p == 2 and nb + 1 < NB and nb >= 2:
                emit_q_blk(nb + 1, only_o=0)
            if p == 6 and nb + 1 < NB and nb >= 2:
                emit_q_blk(nb + 1, only_o=1)
        # avs for pair gp. av0 is deferred for pairs 0-2 of blocks >= 1
        # (single-buffered bank, cleared by the previous block's divide at
        # p==3); den for pair 0 rides iteration 1 so the previous block's
        # den-sum read clears the single den bank first.
        if nb >= 1 and p == 0:
            emit_av1_den(gp, with_den=False)
        elif nb >= 1 and p == 1:
            psum = blk_tiles[nb][2]
            nc.tensor.matmul(psum, lhsT=vt2(0)[:, :, 2 * P:2 * P + 1],
                             rhs=e2_pend[gp - 1], start=True, stop=False,
                             perf_mode=DR)
            emit_av1_den(gp)
        else:
            emit_av1_den(gp)
        if not defer_av0:
            emit_av0(gp, first=(p == 3 if nb >= 1 else p == 0),
                     last=(p == NPAIR - 1 and nb == 0))
            e2_done(gp)
        if nb >= 1 and p >= 13:
            # tail-deferred av0 for pair p-13 (stop flag on the last one)
            dgp = nb * NPAIR + (p - 13)
            emit_av0(dgp, first=False, last=(p == NPAIR - 1))
            e2_done(dgp)
        emit_scores(gp + 2, 1)
        if p == NPAIR - 1:
            # div_a emitted before the next block's first den matmul can
            # touch the single-buffered ps_sum bank
            state["pend"] = (*blk_tiles.pop(nb), nb)
            state["bc_prev"] = emit_div_a(state["pend"])
    emit_div_b(state["pend"], state["bc_prev"], 0)
    emit_div_b(state["pend"], state["bc_prev"], 1)


def build_nc() -> bass.Bass:
    nc = bacc.Bacc("TRN2", target_bir_lowering=False, debug=False)
    x = nc.dram_tensor("x", [C, N], F32, kind="ExternalInput")
    wqkvT = nc.dram_tensor("wqkvT", [P, 2, 3 * C], F32, kind="ExternalInput")
    bqkv = nc.dram_tensor("bqkv", [3 * C], F32, kind="ExternalInput")
    bproj = nc.dram_tensor("bproj", [C], F32, kind="ExternalInput")
    gamma = nc.dram_tensor("gamma", [C], F32, kind="ExternalInput")
    beta = nc.dram_tensor("beta", [C], F32, kind="ExternalInput")
    gmat = nc.dram_tensor("gmat", [P, P], F32, kind="ExternalInput")
    out = nc.dram_tensor("out", [C, N], F32, kind="ExternalOutput")
    with tile.TileContext(nc) as tc:
        emit_kernel(tc, out.ap(), x.ap(), wqkvT.ap(), bqkv.ap(),
                    bproj.ap(), gamma.ap(), beta.ap(), gmat.ap())
    nc.compile()
    return nc


_NC_CACHE: list = []


def _in_maps(x, gamma, beta, w_qkv, b_qkv, w_proj, b_proj):
    f = lambda a: np.ascontiguousarray(np.asarray(a, dtype=np.float32))
    xs = f(x).reshape(B, C, N)
    w_qkv = np.asarray(w_qkv, dtype=np.float64)
    w_proj = np.asarray(w_proj, dtype=np.float64)
    b_qkv = np.asarray(b_qkv, dtype=np.float64)
    b_proj = np.asarray(b_proj, dtype=np.float64)
    # fuse w_proj into the v projection; its bias rides into bproj (softmax
    # rows sum to 1, so a constant vp offset is a constant output offset)
    w_fused = np.concatenate(
        [w_qkv[0:2 * C], w_proj @ w_qkv[2 * C:3 * C]], axis=0)
    bp_eff = b_proj + w_proj @ b_qkv[2 * C:3 * C]
    base = {
        "wqkvT": f(w_fused.T.reshape(2, P, 3 * C).transpose(1, 0, 2)),
        "bqkv": f(b_qkv),
        "bproj": f(bp_eff),
        "gamma": f(gamma),
        "beta": f(beta),
        "gmat": _group_mat(),
    }
    return [{**base, "x": np.ascontiguousarray(xs[i])} for i in range(B)]


def run_spmd(x, gamma, beta, w_qkv, b_qkv, w_proj, b_proj, **kwargs):
    from concourse.bass_utils import run_bass_kernel_spmd

    if not _NC_CACHE:
        _NC_CACHE.append(build_nc())
    nc = _NC_CACHE[0]
    maps = _in_maps(x, gamma, beta, w_qkv, b_qkv, w_proj, b_proj)
    res = run_bass_kernel_spmd(nc, maps, core_ids=list(range(B)), **kwargs)
    out = np.stack([res.results[i]["out"] for i in range(B)])
    return out.reshape(B, C, H, W), res


def kernel(x, gamma, beta, w_qkv, b_qkv, w_proj, b_proj) -> np.ndarray:
    out, _ = run_spmd(x, gamma, beta, w_qkv, b_qkv, w_proj, b_proj)
    return out


# revision 34
# speedup vs baseline: 1.0456x; 1.0456x over previous
"""AttentionBlock (GroupNorm + single-head self-attention + proj + residual)
on 8 TRN2 NeuronCores. Data-parallel over batch: core i handles sample i.

Reference computation per sample (C=256, H=W=64, N=H*W=4096, G=32 groups):
  h    = groupnorm(x) * gamma + beta
  qkv  = w_qkv @ h + b_qkv              (1x1 conv == channel matmul)
  attn = softmax(q^T k / sqrt(C))       (N x N, never materialized in HBM)
  out  = x + w_proj @ (v @ attn^T) + b_proj

v10 structure (vs the 257us baseline):
  - w_proj folded into the v projection on the host: vp = (w_proj@w_v) h,
    so attn@v directly produces projected channels; the 32 proj matmuls and
    the bf16 att tiles disappear. out = x + (E vp)/den + bp_eff.
  - GroupNorm folded into the qkv weights on device (w' = w * sc per input
    channel), so x casts to fp8 on arrival and no h tensor is materialized.
  - bias algebra: k needs NO bias (a per-column score offset cancels in
    softmax); vp needs NO tensor bias (a constant vp offset rides through
    softmax into the output bias: bp_eff = bproj + wproj@bv + W_vp@sh).
    Only q keeps a [P,1] bias.
  - scores live in [P, 2, NBLK] two-bank psum pair-tiles, so the softmax
    exp is ONE 1068ns ACT instruction per m-pair (vs 2x679) and ALL exps
    fit on ACT (~17.1us/block ~ the PE's 17.2us pair stream). The DVE
    stays nearly idle -> no concurrent ACT+DVE clock throttle (which
    drops the chip from 2.4 to 2.0GHz and can latch), no idle wake-ups.
  - PSUM: score pairs 2x2 banks + av0 1 + av1 2 + den 1 = 8. av0 is
    single-buffered: its pair 0-2 matmuls are deferred to iterations
    13-15 (accumulation order is commutative; start flag rides pair 3,
    stop rides the deferred pair 2), clearing the previous block's
    divide without a bubble. qkv borrows score pair-tiles.
  - q projections for blocks 2..7 are deferred into attention blocks
    1..6 (copies on the idle DVE); divide/bias/residual also DVE.
  - x stays resident in SBUF for the residual (no 4MB re-read).
"""

import sys

for _p in ("/opt/trn_rl_repo", "/opt/pypackages"):
    if _p not in sys.path:
        sys.path.append(_p)

from contextlib import ExitStack

import numpy as np

import concourse.bass as bass
import concourse.tile as tile
from concourse import bacc, mybir
from concourse._compat import with_exitstack

B, C, H, W = 8, 256, 64, 64
N = H * W          # 4096
G = 32             # groups
GS = C // G        # 8 channels per group
EPS = 1e-5
P = 128
NCT = C // P       # 2 channel tiles
NBLK = 512         # attention n-block width
NB = N // NBLK     # 8
NM = N // P        # 32 m-tiles
NPAIR = NM // 2    # 16 m-pairs per block
NGP = NB * NPAIR   # 128 global pairs
SCALE = 1.0 / np.sqrt(np.float32(C))  # 1/16
WARMUP_MM = 56      # fp32 gmat matmuls to keep PE's HAM clock-gate warm

F32 = mybir.dt.float32
BF16 = mybir.dt.bfloat16
FP8 = mybir.dt.float8e4
DR = mybir.MatmulPerfMode.DoubleRow
AF = mybir.ActivationFunctionType
ALU = mybir.AluOpType


def _group_mat() -> np.ndarray:
    """A[c, c'] = 1/GS if c and c' are in the same group (within a 128-chan
    tile); A^T @ t group-averages per-channel stats in one PE matmul."""
    a = np.zeros((P, P), np.float32)
    for g in range(P // GS):
        a[g * GS:(g + 1) * GS, g * GS:(g + 1) * GS] = 1.0 / GS
    return a


def _col(ap_1d, lo, hi):
    """Slice a 1-D DRAM AP into a [hi-lo, 1] AP (partition dim x 1)."""
    sl = ap_1d[lo:hi]
    return bass.AP(tensor=sl.tensor, offset=sl.offset, ap=[*sl.ap, [1, 1]])


@with_exitstack
def emit_kernel(ctx: ExitStack, tc: tile.TileContext, out_d, x_d, wqkvT_d,
                bqkv_d, bproj_d, gamma_d, beta_d, gmat_d):
    nc = tc.nc

    big = ctx.enter_context(tc.tile_pool(name="big", bufs=1))
    small = ctx.enter_context(tc.tile_pool(name="small", bufs=1))
    work = ctx.enter_context(tc.tile_pool(name="work", bufs=4))
    workd = ctx.enter_context(tc.tile_pool(name="workd", bufs=3))
    work2 = ctx.enter_context(tc.tile_pool(name="work2", bufs=3))
    tdiv = ctx.enter_context(tc.tile_pool(name="tdiv", bufs=4))
    stage = ctx.enter_context(tc.tile_pool(name="stage", bufs=4))
    ps_sp = ctx.enter_context(tc.tile_pool(name="ps_sp", bufs=2, space="PSUM"))
    ps_av0 = ctx.enter_context(tc.tile_pool(name="ps_av0", bufs=1, space="PSUM"))
    ps_av1 = ctx.enter_context(tc.tile_pool(name="ps_av1", bufs=2, space="PSUM"))
    ps_sum = ctx.enter_context(tc.tile_pool(name="ps_sum", bufs=1, space="PSUM"))

    def ps_pair(name):
        return ps_sp.tile([P, 2, NBLK], F32, tag="sp", name=name)

    # ---- gmat first: its DVE copy feeds PE warmup matmuls that keep the
    # HAM clock-gate warm while x loads / groupnorm stats run ----
    gmat_f = small.tile([P, P], F32, tag="gmatf")
    nc.sync.dma_start(gmat_f, gmat_d[:, :])
    gmat_sb = small.tile([P, P], F32, tag="gmat")
    nc.vector.tensor_copy(gmat_sb, gmat_f)
    for w in range(WARMUP_MM):
        pw = ps_pair(f"warm{w}")
        nc.tensor.matmul(pw[:, 0, 0:P], lhsT=gmat_sb, rhs=gmat_sb,
                         start=True, stop=True)

    # ---- constants / weights to SBUF ----
    gamma_t = []
    beta_t = []
    bp_t = []
    for ct in range(NCT):
        gt = small.tile([P, 1], F32, tag=f"gamma{ct}")
        nc.sync.dma_start(gt, _col(gamma_d, ct * P, (ct + 1) * P))
        gamma_t.append(gt)
        bt = small.tile([P, 1], F32, tag=f"beta{ct}")
        nc.sync.dma_start(bt, _col(beta_d, ct * P, (ct + 1) * P))
        beta_t.append(bt)
        t = small.tile([P, 1], F32, tag=f"bp{ct}")
        nc.sync.dma_start(t, _col(bproj_d, ct * P, (ct + 1) * P))
        bp_t.append(t)
    bq_in = []
    for o in range(NCT):  # only q's bias survives the softmax algebra
        t = small.tile([P, 1], F32, tag=f"bq{o}")
        nc.sync.dma_start(t, _col(bqkv_d, o * P, (o + 1) * P))
        bq_in.append(t)

    eps_t = small.tile([P, 1], F32, tag="eps")
    nc.vector.memset(eps_t, float(EPS))
    # preload the Sqrt act table while the engines boot (Sqrt and Exp live
    # in different table sets; each implicit load costs 1.28us on ACT)
    sqrt_dummy = small.tile([P, 1], F32, tag="sqrt_dummy")
    nc.scalar.activation(sqrt_dummy, eps_t, AF.Sqrt, bias=eps_t)

    # ---- load x (3-way split across the SP / GPSIMD / ACT DGE queues);
    # bn_stats (DVE) + fp8 cast (ACT) interleave with chunk arrival. The
    # scalar queue takes the LAST 5 chunk indices so stats emission order
    # matches arrival order, and wqf (0.79MB, needed only at fold time
    # ~30us) loads after them. ----
    x_sb = []
    stats_t = []
    for ct in range(NCT):
        xt = big.tile([P, N], F32, tag=f"x{ct}", name=f"x{ct}")
        x_sb.append(xt)
        stats_t.append(small.tile([P, NB, 6], F32, tag=f"bnst{ct}",
                                  name=f"bnst{ct}"))
    x8 = big.tile([P, 2, N], FP8, tag="x8")
    qs_order = ([nc.sync, nc.gpsimd] * 6)[:11] + [nc.scalar] * 5
    for j in range(NB):
        for ct in range(NCT):
            eng = qs_order[2 * j + ct]
            eng.dma_start(x_sb[ct][:, j * NBLK:(j + 1) * NBLK],
                          x_d[ct * P:(ct + 1) * P, j * NBLK:(j + 1) * NBLK])
    wqf = small.tile([P, 2, 3 * C], F32, tag="wqkvTf", name="wqf")
    nc.scalar.dma_start(wqf, wqkvT_d[:, :, :])
    for j in range(NB):
        for ct in range(NCT):
            csl = slice(j * NBLK, (j + 1) * NBLK)
            nc.vector.bn_stats(stats_t[ct][:, j, :], x_sb[ct][:, csl])
            nc.scalar.copy(x8[:, ct, csl], x_sb[ct][:, csl])

    # ---- GN stats -> per-channel scale/shift (h = x*sc + sh) ----
    scale_sh = []
    for ct in range(NCT):
        mv = small.tile([P, 2], F32, tag=f"mv{ct}")
        nc.vector.bn_aggr(mv, stats_t[ct])
        # t = [mean_c, E[x^2]_c]
        t = small.tile([P, 2], F32, tag=f"t{ct}")
        nc.vector.tensor_copy(t[:, 0:1], mv[:, 0:1])
        nc.vector.tensor_mul(t[:, 1:2], mv[:, 0:1], mv[:, 0:1])
        nc.vector.tensor_add(t[:, 1:2], t[:, 1:2], mv[:, 1:2])
        # group-average + broadcast back to channels via PE
        psg = ps_pair(f"psg{ct}")
        nc.tensor.matmul(psg[:, 0, 0:2], lhsT=gmat_sb, rhs=t,
                         start=True, stop=True)
        g_sb = small.tile([P, 2], F32, tag=f"g{ct}")
        nc.vector.tensor_copy(g_sb, psg[:, 0, 0:2])
        # scale = gamma * rsqrt(var + eps);  shift = beta - group_mean * scale
        tmp = small.tile([P, 1], F32, tag=f"tmp{ct}")
        sc = small.tile([P, 1], F32, tag=f"sc{ct}")
        sh = small.tile([P, 1], F32, tag=f"sh{ct}")
        nc.vector.tensor_mul(tmp, g_sb[:, 0:1], g_sb[:, 0:1])
        nc.vector.tensor_tensor(tmp, g_sb[:, 1:2], tmp, ALU.subtract)  # var
        nc.scalar.activation(tmp, tmp, AF.Sqrt, bias=eps_t)
        nc.vector.reciprocal(tmp, tmp)                                 # rstd
        nc.vector.tensor_mul(sc, tmp, gamma_t[ct])
        nc.vector.tensor_mul(tmp, g_sb[:, 0:1], sc)
        nc.vector.tensor_tensor(sh, beta_t[ct], tmp, ALU.subtract)
        scale_sh.append((sc, sh))
    # switch the ACT table back to Exp now, off the critical path, so the
    # first softmax exp doesn't pay the 1.28us implicit load
    nc.scalar.activation(sqrt_dummy, eps_t, AF.Exp, scale=1.0)

    # ---- fold GN into the weights: wq8[c,:] = wqf[c,:] * sc[c] in fp8;
    # q bias' = W_q @ sh + b_q and bp_eff += W_vp @ sh via tiny matmuls ----
    wq8 = small.tile([P, 2, 3 * C], FP8, tag="wqkvT8", name="wq8")
    for ct in range(NCT):
        nc.vector.tensor_scalar(wq8[:, ct, :], wqf[:, ct, :], scale_sh[ct][0],
                                None, op0=ALU.mult)
    bias_q = []
    bp_eff = []
    for o in range(NCT):
        psb = ps_pair(f"psbq{o}")
        for ct in range(NCT):
            nc.tensor.matmul(psb[:, 0, 0:1],
                             lhsT=wqf[:, ct, o * P:(o + 1) * P],
                             rhs=scale_sh[ct][1], start=(ct == 0),
                             stop=(ct == NCT - 1))
        bj = small.tile([P, 1], F32, tag=f"biasq{o}")
        nc.vector.tensor_add(bj, psb[:, 0, 0:1], bq_in[o])
        bias_q.append(bj)
    for o in range(NCT):
        psb = ps_pair(f"psbp{o}")
        for ct in range(NCT):
            nc.tensor.matmul(
                psb[:, 0, 0:1],
                lhsT=wqf[:, ct, 2 * C + o * P:2 * C + (o + 1) * P],
                rhs=scale_sh[ct][1], start=(ct == 0), stop=(ct == NCT - 1))
        bj = small.tile([P, 1], F32, tag=f"bpe{o}")
        nc.vector.tensor_add(bj, psb[:, 0, 0:1], bp_t[o])
        bp_eff.append(bj)

    # ---- qkv projections (GN pre-folded, so rhs is x8 directly). q/k land
    # in fp8 [128, 2, N] (channel-half on the middle dim) and vp in fp8
    # m-pair-interleaved [128, 2, 272] tiles so the attention matmuls can use
    # fp8 DoubleRow (2 values/PE-cell -> one matmul contracts 256). vp is the
    # w_proj-fused v projection; col 256 = ones (softmax denominators). ----
    q2 = big.tile([P, 2, N], FP8, tag="q2")
    k2 = big.tile([P, 2, N], FP8, tag="k2")
    VTW = 272
    vt_lo = big.tile([P, NM // 4, 2, VTW], FP8, tag="vt0", name="vt_lo")
    vt_hi = big.tile([P, NM // 4, 2, VTW], FP8, tag="vt1", name="vt_hi")
    # pre-fill the ones columns once (strided memsets, off-critical-path)
    nc.gpsimd.memset(vt_lo[:, :, :, C:C + 1], 1.0)
    nc.gpsimd.memset(vt_hi[:, :, :, C:C + 1], 1.0)

    def vt2(pair):
        return (vt_lo[:, pair] if pair < NM // 4
                else vt_hi[:, pair - NM // 4])

    def emit_q_blk(blk, only_o=None, on_act=False):
        """q projection for block blk: 2 matmuls + 2 biased copies.
        Deferrable (per channel-half) to just before block blk needs q2."""
        bsl = slice(blk * NBLK, (blk + 1) * NBLK)
        ps = ps_pair(f"q{blk}_{only_o}")
        for o in range(NCT):
            if only_o is not None and o != only_o:
                continue
            half = 0 if only_o is not None else o
            nc.tensor.matmul(
                ps[:, half], lhsT=wq8[:, :, o * P:(o + 1) * P],
                rhs=x8[:, :, bsl], start=True, stop=True, perf_mode=DR)
            if on_act:
                nc.scalar.activation(q2[:, o, bsl], ps[:, half], AF.Identity,
                                     bias=bias_q[o], scale=1.0)
            else:
                nc.vector.tensor_scalar_add(q2[:, o, bsl], ps[:, half],
                                            bias_q[o])

    def emit_kv_blk(blk):
        """k + vp projections for block blk (biases cancel / fold away, so
        all copies are plain psum->fp8 casts; DVE takes most, ACT the
        rest within its exp-stream slack)."""
        bsl = slice(blk * NBLK, (blk + 1) * NBLK)
        psk = ps_pair(f"k{blk}")
        for o in range(NCT):
            nc.tensor.matmul(
                psk[:, o], lhsT=wq8[:, :, C + o * P:C + (o + 1) * P],
                rhs=x8[:, :, bsl], start=True, stop=True, perf_mode=DR)
            nc.vector.tensor_copy(k2[:, o, bsl], psk[:, o])
        psv = ps_pair(f"v{blk}")
        for i in range(4):
            m = 4 * blk + i
            sl = psv[:, i // 2, (i % 2) * C:(i % 2) * C + C]
            nc.tensor.matmul(
                sl, lhsT=x8[:, :, m * P:(m + 1) * P],
                rhs=wq8[:, :, 2 * C:3 * C],
                start=True, stop=True, perf_mode=DR)
            dst = vt2(m // 2)[:, m % 2]
            nc.vector.tensor_copy(dst[:, 0:C], sl)

    # ---- softmax divide + output helpers (all on the idle DVE/Pool) ----
    def emit_div_a(pend):
        pav0, pav1, psum, nb = pend
        sums_sb = work2.tile([1, NBLK], F32, tag="sums")
        nc.vector.tensor_copy(sums_sb, psum)
        bc2 = work2.tile([P, NBLK], F32, tag="bc2")
        nc.gpsimd.partition_broadcast(bc2, sums_sb)
        bc_sb = work2.tile([P, NBLK], F32, tag="bc")
        nc.vector.reciprocal_approx_fast(bc_sb, bc2)
        return bc_sb

    def emit_div_b(pend, bc_sb, o):
        """One output channel-half: divide + bias + residual + store."""
        pav = pend[o]
        nb = pend[3]
        nsl = slice(nb * NBLK, (nb + 1) * NBLK)
        t = tdiv.tile([P, NBLK], F32, tag="t")
        nc.vector.tensor_mul(t, pav, bc_sb)
        st = stage.tile([P, NBLK], F32, tag="st")
        nc.vector.scalar_tensor_tensor(st, t, bp_eff[o], x_sb[o][:, nsl],
                                       op0=ALU.add, op1=ALU.add)
        eng = nc.sync if o == 0 else nc.gpsimd
        eng.dma_start(out_d[o * P:(o + 1) * P, nsl], st)

    # ---- global software-pipelined attention loop over gp = nb*16 + p.
    # Iteration gp emits: the single ACT exp for pair gp+1 (runs while the
    # PE works), score matmuls for gp+2 into a fresh pair-tile (half 0
    # early, half 1 after the avs), and the av matmuls for gp (exp'd last
    # iteration -> a full pair-period of exp slack). av0 is single-
    # buffered: pairs 0-2 of each block nb>=1 are deferred to iterations
    # 13-15, after the previous block's av0 divide has read the bank. ----
    ps_m = {}
    e2_pend = {}
    blk_tiles = {}

    def emit_scores(gp, half):
        if gp >= NGP:
            return
        nb, p = divmod(gp, NPAIR)
        m = 2 * p + half
        if half == 0:
            ps_m[gp] = ps_pair(f"s{gp}")
        nc.tensor.matmul(ps_m[gp][:, half],
                         lhsT=k2[:, :, m * P:(m + 1) * P],
                         rhs=q2[:, :, nb * NBLK:(nb + 1) * NBLK],
                         start=True, stop=True, perf_mode=DR)

    def emit_exps(gp):
        if gp >= NGP:
            return
        nb, p = divmod(gp, NPAIR)
        pool = workd if (nb >= 1 and p <= 2) else work
        e2 = pool.tile([P, 2, NBLK], FP8, tag="e", name=f"e{gp}")
        nc.scalar.activation(e2, ps_m.pop(gp), AF.Exp, scale=float(SCALE))
        e2_pend[gp] = e2

    def emit_av0(gp, first, last):
        nb, p = divmod(gp, NPAIR)
        pav0 = blk_tiles[nb][0]
        nc.tensor.matmul(pav0, lhsT=vt2(p)[:, :, 0:P], rhs=e2_pend[gp],
                         start=first, stop=last, perf_mode=DR)

    def emit_av1_den(gp, with_den=True):
        nb, p = divmod(gp, NPAIR)
        _, pav1, psum = blk_tiles[nb]
        first, last = (p == 0), (p == NPAIR - 1)
        vtp = vt2(p)
        nc.tensor.matmul(pav1, lhsT=vtp[:, :, P:2 * P], rhs=e2_pend[gp],
                         start=first, stop=last, perf_mode=DR)
        if with_den:
            nc.tensor.matmul(psum, lhsT=vtp[:, :, 2 * P:2 * P + 1],
                             rhs=e2_pend[gp], start=first, stop=last,
                             perf_mode=DR)

    def e2_done(gp):
        e2_pend.pop(gp)

    def new_blk_tiles(nb):
        pav0 = ps_av0.tile([P, NBLK], F32, tag="av0", name=f"av0_{nb}")
        pav1 = ps_av1.tile([P, NBLK], F32, tag="av1", name=f"av1_{nb}")
        psum = ps_sum.tile([1, NBLK], F32, tag="sum", name=f"sum_{nb}")
        blk_tiles[nb] = (pav0, pav1, psum)

    # Fused phase gating: kv block b unlocks k2 m-tiles < 4(b+1) and vt
    # pairs < 2(b+1); at iteration gp of block 0 the scores reach m-tile
    # 2*gp+5 and the avs read vt pair gp, both covered once kv blocks
    # <= gp/2 + 2 are in.
    emit_q_blk(0, on_act=True)
    emit_kv_blk(0)
    emit_kv_blk(1)
    emit_scores(0, 0)
    emit_scores(0, 1)
    emit_exps(0)
    emit_scores(1, 0)
    emit_scores(1, 1)
    emit_q_blk(1, on_act=True)

    state = {"pend": None, "bc_prev": None}
    for it in range(NGP + 1):
        # iteration `it`: exps(it+1), scores(it+2), avs(it-1) -- the av
        # matmuls trail the exp by two iterations, so e2 and the score
        # pair-tile bank are always ready a full iteration before use.
        snb, sp = divmod(it, NPAIR)
        if snb == 0 and it < NGP:
            if sp % 2 == 0 and 2 + sp // 2 < NB:
                emit_kv_blk(2 + sp // 2)
            if sp == 12:
                emit_q_blk(2)
        emit_exps(it + 1)
        emit_scores(it + 2, 0)
        agp = it - 1
        if agp >= 0:
            nb, p = divmod(agp, NPAIR)
            if nb not in blk_tiles:
                new_blk_tiles(nb)
            if nb > 0:
                if p == 3:
                    emit_div_b(state["pend"], state["bc_prev"], 0)
                if p == 4:
                    emit_div_b(state["pend"], state["bc_prev"], 1)
                if p == 2 and nb + 1 < NB and nb >= 2:
                    emit_q_blk(nb + 1, only_o=0)
                if p == 6 and nb + 1 < NB and nb >= 2:
                    emit_q_blk(nb + 1, only_o=1)
            # avs for pair agp. av0 is deferred for pairs 0-2 of blocks
            # >= 1 (single-buffered bank, cleared by the previous block's
            # divide at p==3); den for pair 0 rides the next iteration so
            # the previous block's den-sum read clears the den bank first.
            defer_av0 = (nb >= 1 and p <= 2)
            if nb >= 1 and p == 0:
                emit_av1_den(agp, with_den=False)
            elif nb >= 1 and p == 1:
                psum = blk_tiles[nb][2]
                nc.tensor.matmul(psum, lhsT=vt2(0)[:, :, 2 * P:2 * P + 1],
                                 rhs=e2_pend[agp - 1], start=True,
                                 stop=False, perf_mode=DR)
                emit_av1_den(agp)
            else:
                emit_av1_den(agp)
            if not defer_av0:
                emit_av0(agp, first=(p == 3 if nb >= 1 else p == 0),
                         last=(p == NPAIR - 1 and nb == 0))
                e2_done(agp)
            if nb >= 1 and p >= 13:
                # tail-deferred av0 for pair p-13 (stop on the last one)
                dgp = nb * NPAIR + (p - 13)
                emit_av0(dgp, first=False, last=(p == NPAIR - 1))
                e2_done(dgp)
        emit_scores(it + 2, 1)
        if agp >= 0 and agp % NPAIR == NPAIR - 1:
            # div_a emitted before the next block's first den matmul can
            # touch the single-buffered ps_sum bank
            state["pend"] = (*blk_tiles.pop(agp // NPAIR), agp // NPAIR)
            state["bc_prev"] = emit_div_a(state["pend"])
    emit_div_b(state["pend"], state["bc_prev"], 0)
    emit_div_b(state["pend"], state["bc_prev"], 1)


def build_nc() -> bass.Bass:
    nc = bacc.Bacc("TRN2", target_bir_lowering=False, debug=False)
    x = nc.dram_tensor("x", [C, N], F32, kind="ExternalInput")
    wqkvT = nc.dram_tensor("wqkvT", [P, 2, 3 * C], F32, kind="ExternalInput")
    bqkv = nc.dram_tensor("bqkv", [3 * C], F32, kind="ExternalInput")
    bproj = nc.dram_tensor("bproj", [C], F32, kind="ExternalInput")
    gamma = nc.dram_tensor("gamma", [C], F32, kind="ExternalInput")
    beta = nc.dram_tensor("beta", [C], F32, kind="ExternalInput")
    gmat = nc.dram_tensor("gmat", [P, P], F32, kind="ExternalInput")
    out = nc.dram_tensor("out", [C, N], F32, kind="ExternalOutput")
    with tile.TileContext(nc) as tc:
        emit_kernel(tc, out.ap(), x.ap(), wqkvT.ap(), bqkv.ap(),
                    bproj.ap(), gamma.ap(), beta.ap(), gmat.ap())
    nc.compile()
    return nc


_NC_CACHE: list = []


def _in_maps(x, gamma, beta, w_qkv, b_qkv, w_proj, b_proj):
    f = lambda a: np.ascontiguousarray(np.asarray(a, dtype=np.float32))
    xs = f(x).reshape(B, C, N)
    w_qkv = np.asarray(w_qkv, dtype=np.float64)
    w_proj = np.asarray(w_proj, dtype=np.float64)
    b_qkv = np.asarray(b_qkv, dtype=np.float64)
    b_proj = np.asarray(b_proj, dtype=np.float64)
    # fuse w_proj into the v projection; its bias rides into bproj (softmax
    # rows sum to 1, so a constant vp offset is a constant output offset)
    w_fused = np.concatenate(
        [w_qkv[0:2 * C], w_proj @ w_qkv[2 * C:3 * C]], axis=0)
    bp_eff = b_proj + w_proj @ b_qkv[2 * C:3 * C]
    base = {
        "wqkvT": f(w_fused.T.reshape(2, P, 3 * C).transpose(1, 0, 2)),
        "bqkv": f(b_qkv),
        "bproj": f(bp_eff),
        "gamma": f(gamma),
        "beta": f(beta),
        "gmat": _group_mat(),
    }
    return [{**base, "x": np.ascontiguousarray(xs[i])} for i in range(B)]


def run_spmd(x, gamma, beta, w_qkv, b_qkv, w_proj, b_proj, **kwargs):
    from concourse.bass_utils import run_bass_kernel_spmd

    if not _NC_CACHE:
        _NC_CACHE.append(build_nc())
    nc = _NC_CACHE[0]
    maps = _in_maps(x, gamma, beta, w_qkv, b_qkv, w_proj, b_proj)
    res = run_bass_kernel_spmd(nc, maps, core_ids=list(range(B)), **kwargs)
    out = np.stack([res.results[i]["out"] for i in range(B)])
    return out.reshape(B, C, H, W), res


def kernel(x, gamma, beta, w_qkv, b_qkv, w_proj, b_proj) -> np.ndarray:
    out, _ = run_spmd(x, gamma, beta, w_qkv, b_qkv, w_proj, b_proj)
    return out
